# revision 3
# baseline (speedup 1.0000x reference)
"""Trainium2 Bass kernel for nn_MoE_25005299597538 (moe_routing).

Strategy: expert-parallel with host-side routing (the gate is 0.01% of the
FLOPs; the reference's fp32 top-5 selection is reproduced exactly by an fp64
host gate — verified min p5/p6 gap 1.9e-6 >> fp32 rounding noise).

  host:   w = renorm(top5(softmax(x@gate_W/T)))           [N, E]
          for each expert e: gather its active tokens (count ~5156 of 8192,
          5/8 sparsity) into a padded [C=5248] slab; core e gets expert e's
          weights (bf16) + its gathered tokens (bf16, pre-transposed).
  device: per core: resident bf16 W1/W2/W3 in SBUF (16.8 MB), stream token
          blocks of 512: h1=relu(W1x+b1); h2=relu(W2h1+b2); y=w*(W3h2);
          all matmuls bf16 (1 PE cycle/row — same rate as fp32r but half the
          DMA traffic and no min-free-size penalty). Output f32.
  host:   scatter-add the disjoint (expert, token) outputs into y[N, O],
          plus the (sum_e w)*b3 term.

Device compute: 5248 tok * 512 rows = 2.69M PE rows/core vs 4.19M for the
dense all-expert baseline (1.39 ms measured, PE-bound) -> ~0.9 ms target.
No collectives: expert outputs are disjoint row sets, combined on host.
"""

import numpy as np

import concourse.bass as bass
import concourse.tile as tile
import concourse.mybir as mybir
from concourse import bacc

# Problem constants (hardcoded per contract; kernel.py must be self-contained).
N, D, H, O, E = 8192, 1024, 2048, 1024, 8
CORES = 8
TEMP = float(np.e)
N_ACTIVE = 5
EPS = 1e-8
C_DEFAULT = 5248          # per-expert token capacity (41*128); actual ~5156

F32 = mybir.dt.float32
BF16 = mybir.dt.bfloat16


def build_nc(cap=C_DEFAULT, d=D, h=H, o=O):
    """Per-core Bass program: one expert's 3-layer MLP over `cap` tokens."""
    P = 128
    DC = d // P            # 8  contraction chunks, layer 1
    HC = h // P            # 16 h chunks (L1/L2 out, L2/L3 contraction)
    OW = 512
    OT = o // OW           # 2
    TB = 512               # token block
    NQ = cap // P          # 128-token chunks total (41)
    assert cap % P == 0
    blocks = [(b * TB, TB) for b in range(cap // TB)]
    if cap % TB:
        blocks.append((cap - cap % TB, cap % TB))

    nc = bacc.Bacc(None)

    xg_ext = nc.dram_tensor("xg", [DC, P, cap], BF16, kind="ExternalInput")
    w1_ext = nc.dram_tensor("w1p", [P, HC, DC, P], BF16, kind="ExternalInput")
    w2_ext = nc.dram_tensor("w2p", [P, HC, HC, P], BF16, kind="ExternalInput")
    w3_ext = nc.dram_tensor("w3p", [P, OT, HC, OW], BF16, kind="ExternalInput")
    b1_ext = nc.dram_tensor("b1p", [P, HC], F32, kind="ExternalInput")
    b2_ext = nc.dram_tensor("b2p", [P, HC], F32, kind="ExternalInput")
    wg_ext = nc.dram_tensor("wg", [P, NQ], F32, kind="ExternalInput")
    y_ext = nc.dram_tensor("y", [cap, o], F32, kind="ExternalOutput")

    with tile.TileContext(nc) as tc:
        with (
            tc.tile_pool(name="const", bufs=1) as cpool,
            tc.tile_pool(name="xgs", bufs=2) as xpool,
            tc.tile_pool(name="acts", bufs=3) as apool,
            tc.tile_pool(name="yout", bufs=4) as ypool,
            tc.tile_pool(name="ps_mm", bufs=4, space="PSUM") as mmps,
            tc.tile_pool(name="ps_out", bufs=4, space="PSUM") as outps,
        ):
            # resident weights. SP queue: w1 first so block-0 L1 starts ~15us
            # in; the bigger w2/w3 stream on the Activation HWDGE queue and
            # are ready before block-0 L2/L3 need them.
            w1_sb = cpool.tile([P, HC, DC, P], BF16, tag="w1")
            nc.sync.dma_start(w1_sb[:], w1_ext[:])
            w2_sb = cpool.tile([P, HC, HC, P], BF16, tag="w2")
            nc.scalar.dma_start(w2_sb[:], w2_ext[:])
            w3_sb = cpool.tile([P, OT, HC, OW], BF16, tag="w3")
            nc.scalar.dma_start(w3_sb[:], w3_ext[:])
            b1_sb = cpool.tile([P, HC], F32, tag="b1")
            nc.gpsimd.dma_start(b1_sb[:], b1_ext[:])
            b2_sb = cpool.tile([P, HC], F32, tag="b2")
            nc.gpsimd.dma_start(b2_sb[:], b2_ext[:])
            wg_sb = cpool.tile([P, NQ], F32, tag="wg")
            nc.gpsimd.dma_start(wg_sb[:], wg_ext[:])

            xg_v = xg_ext.rearrange("c p n -> p c n")
            for (t0, tb) in blocks:
                xg_t = xpool.tile([P, DC, tb], BF16, tag="xg")
                nc.sync.dma_start(xg_t[:], xg_v[:, :, t0:t0 + tb])

                # layer 1: h1T[j] = relu(sum_dc W1t(j,dc).T @ xgT(dc) + b1)
                h1T = apool.tile([P, HC, tb], BF16, tag="hact")
                for j in range(HC):
                    ps = mmps.tile([P, tb], F32, tag="mm")
                    for dc in range(DC):
                        nc.tensor.matmul(
                            ps[:], w1_sb[:, j, dc, :], xg_t[:, dc, :],
                            start=(dc == 0), stop=(dc == DC - 1),
                        )
                    nc.scalar.activation(
                        h1T[:, j, :], ps[:],
                        mybir.ActivationFunctionType.Relu,
                        bias=b1_sb[:, j:j + 1],
                    )

                # layer 2: h2T[j2] = relu(sum_k W2t(j2,k).T @ h1T(k) + b2)
                h2T = apool.tile([P, HC, tb], BF16, tag="hact")
                for j2 in range(HC):
                    ps = mmps.tile([P, tb], F32, tag="mm")
                    for k in range(HC):
                        nc.tensor.matmul(
                            ps[:], w2_sb[:, j2, k, :], h1T[:, k, :],
                            start=(k == 0), stop=(k == HC - 1),
                        )
                    nc.scalar.activation(
                        h2T[:, j2, :], ps[:],
                        mybir.ActivationFunctionType.Relu,
                        bias=b2_sb[:, j2:j2 + 1],
                    )

                # layer 3 + routing-weight scale, per 128-token chunk
                for qq in range(tb // P):
                    q = t0 // P + qq
                    for ot in range(OT):
                        psO = outps.tile([P, OW], F32, tag="out")
                        for k in range(HC):
                            nc.tensor.matmul(
                                psO[:], h2T[:, k, qq * P:(qq + 1) * P],
                                w3_sb[:, ot, k, :],
                                start=(k == 0), stop=(k == HC - 1),
                            )
                        yt = ypool.tile([P, OW], F32, tag="yt")
                        nc.vector.tensor_scalar_mul(
                            yt[:], psO[:], wg_sb[:, q:q + 1])
                        nc.gpsimd.dma_start(
                            y_ext[q * P:(q + 1) * P, ot * OW:(ot + 1) * OW],
                            yt[:])
    nc.compile()
    return nc


# ---------------------------------------------------------------------------
# Host side: routing, packing, PJRT runner (jit once, reusable), unshard.
# ---------------------------------------------------------------------------

def route(x, gate_W, gate_b):
    """fp64 gate; reproduces the reference's fp32 top-5 selection exactly
    (verified: min |p5-p6| gap 1.9e-6 >> fp32 matmul noise ~1e-7)."""
    s = (x.astype(np.float64) @ gate_W.astype(np.float64)
         + gate_b.astype(np.float64)) / TEMP
    s -= s.max(axis=-1, keepdims=True)
    p = np.exp(s)
    p /= p.sum(axis=-1, keepdims=True)
    order = np.argsort(-p, axis=-1, kind="stable")   # jax top_k tie-break
    mask = np.zeros_like(p)
    mask[np.arange(p.shape[0])[:, None], order[:, :N_ACTIVE]] = 1.0
    w = p * mask
    w /= (w.sum(axis=-1, keepdims=True) + EPS)
    return w.astype(np.float32)


def pack_inputs(x, gate_W, gate_b, W1, b1, W2, b2, W3, b3, cap=C_DEFAULT):
    """Route on host; per expert e, gather + pre-tile its tokens/weights."""
    P = 128
    n_experts, d, h = W1.shape
    o = W3.shape[2]
    DC, HC = d // P, h // P
    OW = 512
    OT = o // OW
    NQ = cap // P
    f32 = np.float32

    import ml_dtypes
    bf16 = ml_dtypes.bfloat16

    w = route(x, gate_W, gate_b)                      # [N, E]
    in_maps, aux = [], []
    for e in range(n_experts):
        idx = np.nonzero(w[:, e])[0]
        cnt = len(idx)
        assert cnt <= cap, f"expert {e}: {cnt} active tokens > cap {cap}"
        xg = np.zeros((cap, d), f32)
        xg[:cnt] = x[idx]
        # xgT layout [DC, P, C]: (dc, p, c) = xg[c, dc*128+p]
        xgT = np.ascontiguousarray(
            xg.T.reshape(DC, P, cap)).astype(bf16)
        wgv = np.zeros((cap,), f32)
        wgv[:cnt] = w[idx, e]
        wgp = np.ascontiguousarray(wgv.reshape(NQ, P).T)      # [P, NQ]
        # (p, j, dc, ph) = W1[e, dc*128+p, j*128+ph]
        w1p = np.ascontiguousarray(
            W1[e].reshape(DC, P, HC, P).transpose(1, 2, 0, 3)).astype(bf16)
        # (p, j2, k, ph2) = W2[e, k*128+p, j2*128+ph2]
        w2p = np.ascontiguousarray(
            W2[e].reshape(HC, P, HC, P).transpose(1, 2, 0, 3)).astype(bf16)
        # (p, ot, k, ow) = W3[e, k*128+p, ot*512+ow]
        w3p = np.ascontiguousarray(
            W3[e].reshape(HC, P, OT, OW).transpose(1, 2, 0, 3)).astype(bf16)
        b1p = np.ascontiguousarray(b1[e].reshape(HC, P).T).astype(f32)
        b2p = np.ascontiguousarray(b2[e].reshape(HC, P).T).astype(f32)
        in_maps.append(dict(xg=xgT, w1p=w1p, w2p=w2p, w3p=w3p,
                            b1p=b1p, b2p=b2p, wg=wgp))
        aux.append((idx, cnt))
    return in_maps, aux, w


def unshard(res, aux, w, b3, o=O):
    """Scatter-add disjoint per-expert outputs; add the w @ b3 term."""
    y = np.zeros((N, o), np.float32)
    for e, (idx, cnt) in enumerate(aux):
        y[idx] += res[e]["y"][:cnt]
    y += w @ b3.astype(np.float32)                    # b3 is [E, O]
    return y


class SpmdRunner:
    """jit-once PJRT SPMD runner (mirrors bass2jax.run_bass_via_pjrt but
    reusable across calls so the NEFF compile is paid once)."""

    def __init__(self, nc, n_cores):
        import jax
        from jax.sharding import Mesh, PartitionSpec
        from jax.experimental.shard_map import shard_map
        from concourse import bass2jax as b2j

        b2j.install_neuronx_cc_hook()
        self.nc = nc
        self.n_cores = n_cores
        in_names, out_names, out_avals, zero_outs = [], [], [], []
        for alloc in nc.m.functions[0].allocations:
            if not isinstance(alloc, mybir.MemoryLocationSet):
                continue
            name = alloc.memorylocations[0].name
            if alloc.kind == "ExternalInput":
                if not (nc.partition_id_tensor
                        and name == nc.partition_id_tensor.name):
                    in_names.append(name)
            elif alloc.kind == "ExternalOutput":
                out_names.append(name)
                shape = tuple(alloc.tensor_shape)
                dtype = mybir.dt.np(alloc.dtype)
                out_avals.append(jax.core.ShapedArray(shape, dtype))
                zero_outs.append(np.zeros(shape, dtype))
        self.in_names, self.out_names = in_names, out_names
        self.out_avals, self.zero_outs = out_avals, zero_outs
        n_params, n_outs = len(in_names), len(out_names)
        self.n_params = n_params
        all_in_names = list(in_names) + list(out_names)
        partition_name = (nc.partition_id_tensor.name
                          if nc.partition_id_tensor else None)
        if partition_name is not None:
            all_in_names.append(partition_name)

        def _body(*args):
            operands = list(args)
            if partition_name is not None:
                operands.append(b2j.partition_id_tensor())
            outs = b2j._bass_exec_p.bind(
                *operands,
                out_avals=tuple(out_avals),
                in_names=tuple(all_in_names),
                out_names=tuple(out_names),
                lowering_input_output_aliases=(),
                sim_require_finite=True,
                sim_require_nnan=True,
                nc=nc,
            )
            return tuple(outs)

        devices = jax.devices()[:n_cores]
        self.mesh = Mesh(np.asarray(devices), ("core",))
        in_specs = (PartitionSpec("core"),) * (n_params + n_outs)
        out_specs = (PartitionSpec("core"),) * n_outs
        donate = tuple(range(n_params, n_params + n_outs))
        self.fn = jax.jit(
            shard_map(_body, mesh=self.mesh, in_specs=in_specs,
                      out_specs=out_specs, check_rep=False),
            donate_argnums=donate, keep_unused=True,
        )
        self.jax = jax

    def prep(self, in_maps):
        concat = [
            np.concatenate([np.asarray(m[n]) for m in in_maps], axis=0)
            for n in self.in_names
        ]
        return concat

    def zeros(self):
        return [np.zeros((self.n_cores * z.shape[0], *z.shape[1:]), z.dtype)
                for z in self.zero_outs]

    def __call__(self, concat_in, concat_zeros):
        out = self.fn(*concat_in, *concat_zeros)
        self.jax.block_until_ready(out)
        return out

    def time_pipelined(self, concat_in, k=33, reps=2):
        """Dispatch k executions back-to-back (async), block once: the device
        serializes them, so (T_k - T_1)/(k-1) ~= per-execution device time
        with dispatch overhead amortized."""
        import time as _time
        import jax
        from jax.sharding import NamedSharding, PartitionSpec
        sh = NamedSharding(self.mesh, PartitionSpec("core"))
        darrs = [jax.device_put(a, sh) for a in concat_in]
        jax.block_until_ready(darrs)

        def run_batch(n):
            zs = [[jax.device_put(z, sh) for z in self.zeros()]
                  for _ in range(n)]
            for z in zs:
                jax.block_until_ready(z)
            t0 = _time.perf_counter()
            outs = [self.fn(*darrs, *zs[i]) for i in range(n)]
            jax.block_until_ready(outs)
            return _time.perf_counter() - t0

        run_batch(2)  # warm
        t1 = min(run_batch(1) for _ in range(3))
        tk = min(run_batch(k) for _ in range(reps))
        per = (tk - t1) / (k - 1)
        return per, t1, tk

    def split_outs(self, out_arrs):
        res = []
        for c in range(self.n_cores):
            res.append({
                name: np.asarray(out_arrs[i]).reshape(
                    self.n_cores, *self.out_avals[i].shape)[c]
                for i, name in enumerate(self.out_names)
            })
        return res


_CACHE = {}


def _get_runner(cap=C_DEFAULT):
    if cap not in _CACHE:
        nc = build_nc(cap=cap)
        _CACHE[cap] = SpmdRunner(nc, CORES)
    return _CACHE[cap]


def kernel(**inputs):
    x = np.asarray(inputs["x"], dtype=np.float32)
    gate_W = np.asarray(inputs["gate_W"], dtype=np.float32)
    gate_b = np.asarray(inputs["gate_b"], dtype=np.float32)
    W1 = np.asarray(inputs["W1"], dtype=np.float32)
    b1 = np.asarray(inputs["b1"], dtype=np.float32)
    W2 = np.asarray(inputs["W2"], dtype=np.float32)
    b2 = np.asarray(inputs["b2"], dtype=np.float32)
    W3 = np.asarray(inputs["W3"], dtype=np.float32)
    b3 = np.asarray(inputs["b3"], dtype=np.float32)

    # capacity: fixed 5248 covers the reference data (max 5156); if some
    # other input needs more, rebuild at the next 128-multiple.
    w = route(x, gate_W, gate_b)
    maxcnt = int((w > 0).sum(axis=0).max())
    cap = C_DEFAULT if maxcnt <= C_DEFAULT else ((maxcnt + 127) // 128) * 128
    runner = _get_runner(cap)
    in_maps, aux, w = pack_inputs(x, gate_W, gate_b, W1, b1, W2, b2, W3, b3,
                                  cap=cap)
    out = runner(runner.prep(in_maps), runner.zeros())
    res = runner.split_outs(out)
    return unshard(res, aux, w, b3).astype(np.float32)


if __name__ == "__main__":
    print("building...")
    nc = build_nc()
    print("built ok")
